# revision 5
# baseline (speedup 1.0000x reference)
"""GQA attention kernel for Trainium2, tensor-parallel over (batch, kv-head-pair).

Problem: B=2, S=2048, D=2048, 32 q heads / 8 kv heads, head_dim 64,
scores get an additive mask [1,1,S,S] + per-batch graph bias [B,1,S,S].

Sharding: 16 units = (batch 2) x (kv-head-pair 4) over 8 cores; core c handles
batch b = c % 2 and kv heads {2*(c//2), 2*(c//2)+1} (8 q heads). Each core
computes its heads' attention output and its slice of the wo matmul; the host
sums the 4 partial outputs per batch.

The execution environment charges a large flat cost per STATIC instruction
(dynamic loop iterations are nearly free), so the kernel wraps every phase in
For_i hardware loops and keeps the static instruction count minimal:
  - All matmuls are fp32 (self-loading weights; bf16 would add an extra
    InstLdweights per matmul). Stages convert bf16 storage to fp32 on the fly.
  - Matmul lhsT cannot take register offsets, so loop-varying weights are
    staged into fixed SBUF tiles with single batched copies.
  - Phase B: g-loop over 128-row query blocks, inner loop over chunks of 4
    kpos tiles; 4 score matmuls land in a 4-bank PSUM tile that one
    mega-activation exponentiates; probs = exp(qk/8)*exp(mask+bias) via one
    4D-broadcast DVE mul; PV accumulates [attn|denom] via a ones column.
  - Normalization: reciprocal on the PSUM denom row, PE K=1 matmul broadcast,
    two DVE muls, one DMA to a DRAM attn scratch (dynamic-dest SBUF writes
    with partition slices are miscompiled, DRAM roundtrip is correct).
  - PSUM accumulation loops start with a K=1 zero-matmul whose rhs reads the
    previous iteration's last DVE/ACT outputs: an artificial RAW join so the
    reset cannot overtake the prior normalization reads (lhsT is zeros).
Causality needs no special casing: masked tiles have exp(-1e9)=0 probs.
"""

import sys

if "/opt/trn_rl_repo" not in sys.path:
    sys.path.insert(0, "/opt/trn_rl_repo")

import numpy as np
import ml_dtypes
from contextlib import ExitStack

import concourse.bass as bass
import concourse.tile as tile
from concourse import bacc, mybir
from concourse.bass import ds
from concourse.bass_utils import run_bass_kernel_spmd

F32 = mybir.dt.float32
BF16 = mybir.dt.bfloat16
Exp = mybir.ActivationFunctionType.Exp

D = 2048          # model dim
HD = 64           # head dim
NREP = 4          # q heads per kv head
NKVL = 2          # kv heads per core
N_CORES = 8
WCOLS = NREP * 128 + 2 * NKVL * HD  # 768


def build_program(S=2048, loop_n=1):
    T = S // 128      # kpos tiles
    G = S // 128      # q groups
    NSC = S // 512    # projection position chunks
    TC = T // 4       # kpos chunks of 4 tiles

    nc = bacc.Bacc("TRN2", target_bir_lowering=False, debug=False,
                   num_devices=N_CORES)
    x_d = nc.dram_tensor("x", (S, D), BF16, kind="ExternalInput").ap()
    comb_d = nc.dram_tensor("comb", (S, S), BF16, kind="ExternalInput").ap()
    wqkv_d = nc.dram_tensor("wqkv", (D, WCOLS), BF16, kind="ExternalInput").ap()
    wo_d = nc.dram_tensor("wo", (512, D), F32, kind="ExternalInput").ap()
    vt_d = nc.dram_tensor("vt_scratch", (128, S), BF16, kind="Internal").ap()
    attn_d = nc.dram_tensor("attn_scratch", (64, 2, S // 128, 512), F32,
                            kind="Internal").ap()
    expct_d = nc.dram_tensor("expct_scratch", (128, S // 128, S), BF16,
                             kind="Internal").ap()
    y_d = nc.dram_tensor("y", (S, D), F32, kind="ExternalOutput").ap()

    with tile.TileContext(nc) as tc, ExitStack() as ctx:
        def body():
            with ExitStack() as bctx:
                P = bctx.enter_context(tc.tile_pool(name="persist", bufs=1))
                xqT0 = P.tile([64, NREP, S], BF16, name="xqT0")   # [d, r, q]
                xqT1 = P.tile([64, NREP, S], BF16, name="xqT1")
                xkT0 = P.tile([64, T, 128], BF16, name="xkT0")    # [d, t, kpos]
                xkT1 = P.tile([64, T, 128], BF16, name="xkT1")
                xve2 = P.tile([128, T, 130], BF16, name="xve2")   # [kpos, t, V|1]
                zrow = P.tile([1, 512], F32, name="zrow")
                zc65 = P.tile([1, 65], F32, name="zc65")
                one64 = P.tile([128, 64], F32, name="one64")
                nc.vector.memset(zrow, 0.0)
                nc.vector.memset(zc65, 0.0)
                nc.vector.memset(one64, 1.0)
                nc.vector.memset(xve2.rearrange("p t c -> p (t c)"), 0.0)
                nc.vector.memset(xve2[:, :, 64:65], 1.0)
                nc.vector.memset(xve2[:, :, 129:130], 1.0)

                xqT0v = xqT0.rearrange("p r (s c) -> p r s c", c=512)
                xqT1v = xqT1.rearrange("p r (s c) -> p r s c", c=512)
                xqT0g = xqT0.rearrange("p r (g q) -> p r g q", q=128)
                xqT1g = xqT1.rearrange("p r (g q) -> p r g q", q=128)
                xkT0v = xkT0.rearrange("p (s t) q -> p s t q", t=4)
                xkT1v = xkT1.rearrange("p (s t) q -> p s t q", t=4)
                xve2v = xve2.rearrange("p (s t) c -> p s t c", t=4)

                # ---------------- Phase A: QKV projections ----------------
                with tc.tile_pool(name="pa", bufs=1) as pa, \
                     tc.tile_pool(name="psA", bufs=1, space="PSUM") as psA:
                    w3 = pa.tile([128, 16, WCOLS], BF16, name="w3")
                    nc.sync.dma_start(
                        w3, wqkv_d.rearrange("(t p) o -> p t o", p=128))
                    xT = pa.tile([128, 16, S], BF16, name="xT")
                    x4 = x_d.rearrange("s (t c) -> s t c", c=128)
                    comb4 = comb_d.rearrange("s (t c) -> s t c", c=128)
                    with tc.tile_pool(name="cstg", bufs=2) as cstg:
                        with tc.For_i(0, 16) as ti:
                            nc.sync.dma_start_transpose(
                                xT[:, ds(ti, 1), :], x4[:, ds(ti, 1), :])
                            cT = cstg.tile([128, S], BF16, tag="cT",
                                           name="cT")
                            nc.sync.dma_start_transpose(
                                cT, comb4[:, ds(ti, 1), :])
                            eStg = cstg.tile([128, S], BF16, tag="eS",
                                             name="eStg")
                            nc.scalar.activation(eStg, cT, Exp)
                            nc.sync.dma_start(expct_d[:, ds(ti, 1), :], eStg)

                    xTv = xT.rearrange("p t (s c) -> p t s c", c=512)
                    xsc = pa.tile([128, 16, 512], F32, name="xsc")
                    sw = pa.tile([128, WCOLS], F32, name="sw")
                    vts = pa.tile([128, 512], BF16, name="vts")
                    qsh = pa.tile([128, NREP, 512], BF16, name="qsh")
                    ksh = pa.tile([128, 512], BF16, name="ksh")
                    vn = pa.tile([128, 128], BF16, name="vn")
                    vtt4 = vt_d.rearrange("p (s j q) -> p s j q", j=4, q=128)
                    xve2v4 = xve2.rearrange("p (s j) c -> p s j c", j=4)
                    psQ4 = psA.tile([128, NREP, 512], F32, tag="q4", name="q4")
                    psK = psA.tile([128, 512], F32, tag="k", name="k")
                    psV = psA.tile([128, 512], F32, tag="v", name="v")
                    vt3 = vt_d.rearrange("p (s c) -> p s c", c=512)

                    with tc.For_i(0, NSC) as sc:
                        nc.scalar.copy(xsc, xTv[:, :, ds(sc, 1), :])
                        for r in range(NREP):
                            nc.tensor.matmul(psQ4[:, r, :], zrow[:, 0:128],
                                             zrow, start=True, stop=False,
                                             skip_group_check=True)
                        nc.tensor.matmul(psK, zrow[:, 0:128], zrow,
                                         start=True, stop=False,
                                         skip_group_check=True)
                        nc.tensor.matmul(psV, zrow[:, 0:128], zrow,
                                         start=True, stop=False,
                                         skip_group_check=True)
                        with tc.For_i(0, 16) as ti:
                            nc.scalar.copy(sw, w3[:, ds(ti, 1), :])
                            rhs = xsc[:, ds(ti, 1), :]
                            for r in range(NREP):
                                nc.tensor.matmul(
                                    psQ4[:, r, :],
                                    sw[:, r * 128:(r + 1) * 128], rhs,
                                    start=False, stop=True,
                                    skip_group_check=True)
                            nc.tensor.matmul(psK, sw[:, 512:640], rhs,
                                             start=False, stop=True,
                                             skip_group_check=True)
                            nc.tensor.matmul(psV, sw[:, 640:768], rhs,
                                             start=False, stop=True,
                                             skip_group_check=True)
                        # Q: rows 0:64 -> xqT0; rows 64:128 staged then DMA'd
                        nc.scalar.copy(xqT0v[:, :, ds(sc, 1), :],
                                       psQ4[0:64, :, :])
                        nc.scalar.copy(qsh[64:128, :, :], psQ4[64:128, :, :])
                        nc.sync.dma_start(xqT1v[:, :, ds(sc, 1), :],
                                          qsh[64:128, :, :])
                        nc.scalar.copy(
                            xkT0v[:, ds(sc, 1), :, :],
                            psK[0:64, :].rearrange("p (t q) -> p t q", q=128))
                        nc.scalar.copy(ksh[64:128, :], psK[64:128, :])
                        nc.sync.dma_start(
                            xkT1v[:, ds(sc, 1), :, :],
                            ksh[64:128, :].rearrange("p (t q) -> p t q", q=128))
                        nc.vector.tensor_copy(vts, psV)
                        nc.sync.dma_start(vt3[:, ds(sc, 1), :], vts)
                        for j in range(4):
                            nc.sync.dma_start_transpose(
                                vn, vtt4[:, ds(sc, 1), j, :])
                            nc.scalar.copy(xve2v4[:, ds(sc, 1), j, 0:64],
                                           vn[:, 0:64])
                            nc.scalar.copy(xve2v4[:, ds(sc, 1), j, 65:129],
                                           vn[:, 64:128])

                # ---------------- Phase B: attention ----------------
                with tc.tile_pool(name="pb", bufs=1) as pb, \
                     tc.tile_pool(name="psB", bufs=1, space="PSUM") as psB:
                    sS4 = psB.tile([128, 2048], F32, tag="s4", name="s4")
                    oP2 = psB.tile([65, 1024], F32, tag="o2", name="o2")
                    recp = psB.tile([64, 1024], F32, tag="rp", name="rp")
                    eCg = pb.tile([128, T, 128], BF16, name="eCg")
                    eS = pb.tile([128, 2048], F32, name="eS")
                    eT = pb.tile([128, 2048], F32, name="eT")
                    sk0s = pb.tile([64, 4, 128], F32, name="sk0s")
                    sk1s = pb.tile([64, 4, 128], F32, name="sk1s")
                    sv4 = pb.tile([128, 4, 130], F32, name="sv4")
                    sq0 = pb.tile([64, NREP, 128], F32, name="sq0")
                    sq1 = pb.tile([64, NREP, 128], F32, name="sq1")
                    den = pb.tile([128, 1024], F32, name="den")
                    recb = pb.tile([64, 1024], F32, name="recb")
                    shf01 = pb.tile([64, 1024], F32, name="shf01")
                    wo3 = pb.tile([128, NREP, D], F32, name="wo3")
                    nc.sync.dma_start(
                        wo3, wo_d.rearrange("(r p) n -> p r n", p=128))
                    sa = pb.tile([128, NREP, 128], F32, name="sa")
                    sa2 = sa.rearrange("p r q -> p (r q)")
                    ysb = pb.tile([128, D], F32, name="ysb")
                    pY = psB.tile([128, D], F32, tag="s4", name="pY")
                    y3 = y_d.rearrange("(s p) n -> s p n", p=128)
                    attn_v = attn_d.rearrange("r v g c -> v r g c")

                    expCTg = expct_d.rearrange("p t (g q) -> p t g q", q=128)
                    eCg4 = eCg.rearrange("p (s t) q -> p s t q", t=4)
                    eS4 = eS.rearrange("p (t r q) -> p t r q", t=4, r=NREP)
                    eT4 = eT.rearrange("p (t r q) -> p t r q", t=4, r=NREP)

                    with tc.For_i(0, G) as g:
                        nc.sync.dma_start(eCg, expCTg[:, :, ds(g, 1), :])
                        nc.scalar.copy(sq0, xqT0g[:, :, ds(g, 1), :])
                        nc.scalar.copy(sq1, xqT1g[:, :, ds(g, 1), :])
                        # zero-reset oP2; rhs operands create artificial RAW
                        # joins on the previous iteration's last DVE (shf01)
                        # and ACT (recb) writes so the PE reset cannot overtake
                        # the normalization reads of oP2 (lhsT is zeros, so
                        # the result is 0 regardless of rhs values).
                        nc.tensor.matmul(oP2[:, 0:512], zc65,
                                         shf01[0:1, 512:1024],
                                         start=True, stop=False,
                                         skip_group_check=True)
                        nc.tensor.matmul(oP2[:, 512:1024], zc65,
                                         ysb[0:1, 0:512],
                                         start=True, stop=False,
                                         skip_group_check=True)
                        with tc.For_i(0, TC) as tcc:
                            nc.scalar.copy(sk0s, xkT0v[:, ds(tcc, 1), :, :])
                            nc.scalar.copy(sk1s, xkT1v[:, ds(tcc, 1), :, :])
                            nc.scalar.copy(sv4, xve2v[:, ds(tcc, 1), :, :])
                            for kvl in range(2):
                                sks = sk0s if kvl == 0 else sk1s
                                sqs = sq0 if kvl == 0 else sq1
                                for dt in range(4):
                                    nc.tensor.matmul(
                                        sS4[:, dt * 512:(dt + 1) * 512],
                                        sks[:, dt, :],
                                        sqs.rearrange("p r q -> p (r q)"),
                                        start=True, stop=True,
                                        skip_group_check=True)
                                nc.scalar.activation(eS, sS4, Exp)
                                in1 = (eCg4[:, ds(tcc, 1), :, :].unsqueeze(3)
                                       .broadcast_to((128, 1, 4, NREP, 128)))
                                nc.vector.tensor_mul(
                                    eT4.unsqueeze(1), eS4.unsqueeze(1), in1)
                                for dt in range(4):
                                    nc.tensor.matmul(
                                        oP2[:, kvl * 512:(kvl + 1) * 512],
                                        sv4[:, dt, kvl * 65:kvl * 65 + 65],
                                        eT[:, dt * 512:(dt + 1) * 512],
                                        start=False, stop=True,
                                        skip_group_check=True)
                        # normalization
                        nc.vector.reciprocal(oP2[64:65, :], oP2[64:65, :])
                        nc.scalar.copy(den[64:65, :], oP2[64:65, :])
                        for j in range(2):
                            nc.tensor.matmul(recp[:, j * 512:(j + 1) * 512],
                                             one64[64:65, :],
                                             den[64:65, j * 512:(j + 1) * 512],
                                             start=True, stop=True,
                                             skip_group_check=True)
                        nc.scalar.copy(recb, recp)
                        nc.vector.tensor_mul(shf01[:, 0:512],
                                             oP2[0:64, 0:512], recb[:, 0:512])
                        nc.vector.tensor_mul(shf01[:, 512:1024],
                                             oP2[0:64, 512:1024],
                                             recb[:, 512:1024])
                        nc.sync.dma_start(
                            attn_d[:, :, ds(g, 1), :],
                            shf01.rearrange("p (v c) -> p v c", v=2))
                        # fused output projection for this query block
                        nc.sync.dma_start(sa2, attn_v[:, :, ds(g, 1), :])
                        for r in range(NREP):
                            for ch in range(4):
                                nc.tensor.matmul(
                                    pY[:, ch * 512:(ch + 1) * 512],
                                    sa[:, r, :],
                                    wo3[:, r, ch * 512:(ch + 1) * 512],
                                    start=(r == 0), stop=(r == NREP - 1),
                                    skip_group_check=True)
                        nc.scalar.copy(ysb, pY)
                        nc.sync.dma_start(y3[ds(g, 1), :, :], ysb)

        for _rep in range(loop_n):
            body()

    nc.compile()
    return nc


def shard_inputs(x, mask, graph_bias, wq, wk, wv, wo, S=2048):
    """Build the 8 per-core input maps from the full inputs."""
    mask2 = np.asarray(mask, dtype=np.float32).reshape(S, S)
    gb = np.asarray(graph_bias, dtype=np.float32).reshape(2, S, S)
    comb_b = [(mask2 + gb[b]).astype(ml_dtypes.bfloat16) for b in range(2)]
    x = np.asarray(x, dtype=np.float32)
    x_bf = [np.ascontiguousarray(x[b]).astype(ml_dtypes.bfloat16)
            for b in range(2)]
    wq = np.asarray(wq, dtype=np.float32) * 0.125  # fold 1/sqrt(HD) into wq
    wk = np.asarray(wk, dtype=np.float32)
    wv = np.asarray(wv, dtype=np.float32)
    wo = np.asarray(wo, dtype=np.float32)

    in_maps = []
    for c in range(N_CORES):
        b = c % 2
        kvp = c // 2
        kvg = (2 * kvp, 2 * kvp + 1)
        # wqkv cols: [r0(kv0|kv1), r1, r2, r3, K(kv0|kv1), V(kv0|kv1)]
        qcols = []
        for r in range(NREP):
            for kv in kvg:
                h = kv * NREP + r
                qcols.extend(range(h * HD, (h + 1) * HD))
        kcols = []
        for kv in kvg:
            kcols.extend(range(kv * HD, (kv + 1) * HD))
        wqkv = np.concatenate(
            [wq[:, qcols], wk[:, kcols], wv[:, kcols]], axis=1)
        # wo rows: [r, kvl, d] to match attn partitions (kvl*64+d) per r chunk
        orows = []
        for r in range(NREP):
            for kv in kvg:
                h = kv * NREP + r
                orows.extend(range(h * HD, (h + 1) * HD))
        in_maps.append({
            "x": x_bf[b],
            "comb": comb_b[b],
            "wqkv": np.ascontiguousarray(wqkv.astype(ml_dtypes.bfloat16)),
            "wo": np.ascontiguousarray(wo[orows, :]),
        })
    return in_maps


def gather_outputs(results, S=2048):
    y = np.zeros((2, S, D), dtype=np.float32)
    for c in range(N_CORES):
        y[c % 2] += results[c]["y"]
    return y


_PROGRAM_CACHE = {}


def _get_program(S, loop_n=1):
    key = (S, loop_n)
    if key not in _PROGRAM_CACHE:
        _PROGRAM_CACHE[key] = build_program(S=S, loop_n=loop_n)
    return _PROGRAM_CACHE[key]


def kernel(x, mask, graph_bias, wq, wk, wv, wo, start_pos=0):
    import time as _time

    S = x.shape[1]
    nc = _get_program(S)
    in_maps = shard_inputs(x, mask, graph_bias, wq, wk, wv, wo, S=S)
    last = None
    for attempt in range(3):
        try:
            res = run_bass_kernel_spmd(nc, in_maps, core_ids=list(range(N_CORES)))
            return gather_outputs(res.results, S=S)
        except Exception as e:  # noqa: BLE001
            last = e
            _time.sleep(20 * (attempt + 1))
    raise last
